# revision 13
# baseline (speedup 1.0000x reference)
"""LIF spiking-neuron scan on 8 Trainium2 NeuronCores — multi-step DVE ops.

Reference semantics (bit-exact):
    mem_t = v_decay * mem_{t-1} * (1 - spk_{t-1}) + x_t
    spk_t = ((mem_t / (v_th + 1e-8)) - 1 > 0)        # for v_th > 0

Device: ONE fused custom-DVE instruction per BLOCK of up to 10 timesteps
(validated exact on HW up to 12 steps = 3072 elems/partition):
    out = mem[steps t0..t0+K),  in0 = mem[steps t0-1..t0+K-1)
in0 aliases the instruction's own output shifted back one timestep
(FD=256 elements).  The DVE datapath streams elements in order with a
write-to-read latency well under 256 elements, so each step reads the
previous step's freshly-written values.  This cuts the sequential chain
from 100 instructions (~417ns each) to 16 (~267ns/step data-limited).

mem lives in a 60-step ring (contiguous steps -> contiguous SBUF, so the
aliased APs work); the single ring wrap at step 60 is stitched with a
1-step op.  Ring reuse is gated on the Scalar engine's per-block Sign
(mem -> s8 fp8 spike codes, host decode spk = s8 > 0).  Loads alternate
whole blocks between the SP and ACT HWDGE queues; s8 stores are issued
JIT on the SP queue behind its loads.  The final TAILS steps skip the
Sign path entirely: raw f32 mem is stored straight from the ring, gated
only on the DVE, so the kernel tail never waits on ACT.

Sharding: batch dim (64) split 8 ways -> per core [T=100, 8, 4096] =
[128 partitions, 100*256 f32] time-major.  Whole input xb persists in
SBUF (100KB/partition); loads issued upfront, whole blocks alternating
between the SP and ACT HWDGE queues.
"""

import os

import numpy as np

import concourse.bass as bass
import concourse.mybir as mybir
from concourse.bass_utils import run_bass_kernel_spmd

T, B, N = 100, 64, 4096
NCORES = 8
P = 128
BPC = B // NCORES          # batch rows per core
FD = BPC * N // P          # 256 free elems per partition per timestep
F32 = mybir.dt.float32
F8 = mybir.dt.float8e4

EPS = np.float32(1e-8)

BLOCKS = [2, 3, 5] + [10] * 8 + [4, 3, 2, 1]
assert sum(BLOCKS) == T
STARTS = [sum(BLOCKS[:i]) for i in range(len(BLOCKS))]
NBL = len(BLOCKS)
MRING = 60                 # mem ring length in steps
NTAILB = 2                 # final blocks stored as raw f32 mem (skip Sign)
TAILS = sum(BLOCKS[-NTAILB:])

_TRACE = bool(os.environ.get("LIF_TRACE"))
LAST_RUN = None  # BassKernelResults of the most recent run (for test.py)


# ---------------------------------------------------------------------------
# Custom fused DVE op registration (runtime-append to concourse.dve_ops.OPS)
# ---------------------------------------------------------------------------
_REGISTERED = {}


def _register_lif_op(ge_mask: bool):
    """LIF_STEP_(LE|GE): out = (Src0 cmp C0) * Src0 * C1 + Src1."""
    name = "LIF_STEP_GE" if ge_mask else "LIF_STEP_LE"
    if name in _REGISTERED:
        return _REGISTERED[name]
    import concourse.dve_ops as dops
    from concourse.dve_spec import Spec, Src0, Src1, C0, C1, lower
    from concourse.dve_uop import DveOpSpec

    if ge_mask:
        body = (Src0 >= C0) * Src0 * C1 + Src1
        ref = lambda in0, in1, s0, s1, imm2: (
            (in0 * (in0 >= s0).astype(np.float32)).astype(np.float32)
            * np.float32(s1) + in1).astype(np.float32)
    else:
        body = (Src0 <= C0) * Src0 * C1 + Src1
        ref = lambda in0, in1, s0, s1, imm2: (
            (in0 * (in0 <= s0).astype(np.float32)).astype(np.float32)
            * np.float32(s1) + in1).astype(np.float32)
    spec = Spec(body=body, reference=ref)

    row = dops._CUSTOM_DVE_ROW_BASE + len(dops.OPS)
    assert row < 0x20, "custom-DVE opcode rows exhausted"
    shas = {}
    for ver in ("v3", "v4"):
        shas[ver] = DveOpSpec(
            name=name, opcode=row, uops=lower(spec, ver=ver), rd1_en=True
        ).sha(ver)
    op = dops.DveOp(name, spec, subdim=False, uops_sha=shas)
    dops.OPS.append(op)
    dops.CUSTOM_DVE_SPECS[name] = spec
    dops._SUB_OPCODE_FOR_NAME[name] = row
    _REGISTERED[name] = op
    return op


# ---------------------------------------------------------------------------
# Threshold boundary (host-side, exact)
# ---------------------------------------------------------------------------
def _predicate(vth: np.float32):
    c = np.float32(vth + EPS)
    assert c != 0.0, "degenerate threshold"
    one = np.float32(1.0)
    if vth > 0:
        pred = lambda m: (np.float32(np.float32(m) / c) - one) > 0
        increasing = True
    else:
        pred = lambda m: (one - np.float32(np.float32(m) / c)) > 0
        increasing = c < 0
    return pred, increasing


def _f32_key(m) -> int:
    i = int(np.frombuffer(np.float32(m).tobytes(), np.uint32)[0])
    return i ^ 0xFFFFFFFF if i & 0x80000000 else i | 0x80000000


def _key_f32(k: int):
    u = (k & 0x7FFFFFFF) if k & 0x80000000 else (k ^ 0xFFFFFFFF)
    return np.frombuffer(np.uint32(u).tobytes(), np.float32)[0]


def spike_boundary(vth: np.float32):
    """Exact f32 boundary b of the spike predicate.
    spk_is_gt: spk = (mem > b), device no-spike mask = (mem is_le b)."""
    with np.errstate(over="ignore"):
        pred, increasing = _predicate(vth)
        lo_k, hi_k = _f32_key(np.float32(-3.4e38)), _f32_key(np.float32(3.4e38))
        if increasing:
            assert not pred(_key_f32(lo_k)) and pred(_key_f32(hi_k))
            while hi_k - lo_k > 1:
                mid = (lo_k + hi_k) // 2
                if pred(_key_f32(mid)):
                    hi_k = mid
                else:
                    lo_k = mid
            b = _key_f32(lo_k)
            assert not pred(b) and pred(_key_f32(lo_k + 1))
            return b, True
        else:
            assert pred(_key_f32(lo_k)) and not pred(_key_f32(hi_k))
            while hi_k - lo_k > 1:
                mid = (lo_k + hi_k) // 2
                if pred(_key_f32(mid)):
                    lo_k = mid
                else:
                    hi_k = mid
            b = _key_f32(hi_k)
            assert not pred(b) and pred(_key_f32(hi_k - 1))
            return b, False


# ---------------------------------------------------------------------------
# Device program
# ---------------------------------------------------------------------------
def build_program(c2: float, d: float, spk_is_gt: bool) -> bass.Bass:
    lif_op = _register_lif_op(ge_mask=not spk_is_gt)
    nc = bass.Bass("TRN2", target_bir_lowering=False, debug=False,
                   enable_asserts=False)
    x_d = nc.dram_tensor("x", [P, T * FD], F32, kind="ExternalInput")
    m_d = nc.dram_tensor("m8", [P, T * FD], F8, kind="ExternalOutput")
    mt_d = nc.dram_tensor("mtail", [P, TAILS * FD], F32, kind="ExternalOutput")
    nb_d = nc.dram_tensor("nbias", [P, 1], F32, kind="ExternalInput")

    xb = nc.alloc_sbuf_tensor("xb", [P, T * FD], F32)        # whole input
    mb = nc.alloc_sbuf_tensor("mb", [P, MRING * FD], F32)    # mem ring
    s8 = nc.alloc_sbuf_tensor("s8", [P, T * FD], F8)         # spike codes
    cst = nc.alloc_sbuf_tensor("const-lif-bias", [P, 1], F32)
    nc.const_aps.aps[(F32, -float(c2))] = cst.ap()

    xc = [nc.alloc_semaphore(f"xc{b}") for b in range(NBL)]  # chunk loaded
    mrd = nc.alloc_semaphore("mrd")   # DVE produced block (1/blk)
    asg = nc.alloc_semaphore("asg")   # ACT signed block (1/blk)
    bsm = nc.alloc_semaphore("bsm")   # bias const loaded
    stm = nc.alloc_semaphore("stm")   # store completions (16/blk)

    def xsl(b):
        lo, L = STARTS[b] * FD, BLOCKS[b] * FD
        return xb[:, lo:lo + L]

    def msl(s0, n=1):
        # steps [s0, s0+n) in ring coords; caller guarantees no wrap
        p = (s0 % MRING) * FD
        assert (s0 % MRING) + n <= MRING, (s0, n)
        return mb[:, p:p + n * FD]

    # ring-reuse gate: block b overwrites ring cells of steps start-MRING...
    # -> need the block containing those steps signed (asg)
    def reuse_gate(b):
        last_old = STARTS[b] + BLOCKS[b] - 1 - MRING
        if last_old < 0:
            return None
        for j in range(NBL):
            if STARTS[j] <= last_old < STARTS[j] + BLOCKS[j]:
                return j + 1
        raise AssertionError(last_old)

    with nc.Block() as blk:

        # early blocks' s8 stores + the f32 tail ride the ACT queue,
        # balancing it against the SP queue (which carries the late stores)
        ACT_STORES = [b for b in range(NBL - NTAILB) if STARTS[b] < 50]

        @blk.sync
        def _(sync):
            for b in range(0, NBL, 2):       # even blocks on SP queue
                sync.dma_start(xsl(b), x_d[:, STARTS[b] * FD:
                                           (STARTS[b] + BLOCKS[b]) * FD]
                               ).then_inc(xc[b], 16)
            for b in range(NBL - NTAILB):
                if b in ACT_STORES:
                    continue
                lo, L = STARTS[b] * FD, BLOCKS[b] * FD
                sync.wait_ge(asg, b + 1)
                sync.dma_start(m_d[:, lo:lo + L], s8[:, lo:lo + L]
                               ).then_inc(stm, 16)
            sync.wait_ge(stm, 16 * (NBL - NTAILB + 1))

        @blk.scalar
        def _(act):
            for b in range(1, NBL, 2):       # odd blocks on ACT queue
                act.dma_start(xsl(b), x_d[:, STARTS[b] * FD:
                                          (STARTS[b] + BLOCKS[b]) * FD]
                              ).then_inc(xc[b], 16)
            act.wait_ge(bsm, 16)             # bias const loaded
            for b in range(NBL - NTAILB):
                lo, L = STARTS[b] * FD, BLOCKS[b] * FD
                ins = act.activation(
                    s8[:, lo:lo + L],
                    msl(STARTS[b], BLOCKS[b]),
                    mybir.ActivationFunctionType.Sign,
                    bias=-float(c2), scale=1.0,
                )
                ins._wait_ge(mrd, b + 1)
                ins.then_inc(asg, 1)
                p = b - 1                    # store one block behind: the
                if p in ACT_STORES:          # asg wait is already satisfied
                    plo, pL = STARTS[p] * FD, BLOCKS[p] * FD
                    act.wait_ge(asg, p + 1)
                    act.dma_start(m_d[:, plo:plo + pL], s8[:, plo:plo + pL]
                                  ).then_inc(stm, 16)
            # last TAILS steps: raw f32 mem straight from the ring, gated on
            # the DVE only -- skips the Sign/asg chain at the very end
            act.wait_ge(mrd, NBL)
            act.dma_start(mt_d[:, :], msl(T - TAILS, TAILS)
                          ).then_inc(stm, 16)

        @blk.vector
        def _(v):
            for b in range(NBL):
                g = reuse_gate(b)
                if g is not None:
                    v.wait_ge(asg, g)
                s0, K = STARTS[b], BLOCKS[b]
                if b == 0:
                    ins = v.tensor_copy(msl(0), xb[:, 0:FD])
                    ins._wait_ge(xc[0], 16)
                    last = v._custom_dve(
                        lif_op, out=msl(1), in0=msl(0),
                        in1=xb[:, FD:2 * FD], s0=float(c2), s1=float(d))
                elif (s0 - 1) % MRING + K > MRING:
                    # ring wrap between in0 start and out end: stitch 1 step
                    ins = v._custom_dve(
                        lif_op, out=msl(s0), in0=msl(s0 - 1),
                        in1=xb[:, s0 * FD:(s0 + 1) * FD],
                        s0=float(c2), s1=float(d))
                    ins._wait_ge(xc[b], 16)
                    last = ins
                    if K > 1:
                        last = v._custom_dve(
                            lif_op, out=msl(s0 + 1, K - 1),
                            in0=msl(s0, K - 1),
                            in1=xb[:, (s0 + 1) * FD:(s0 + K) * FD],
                            s0=float(c2), s1=float(d))
                else:
                    last = v._custom_dve(
                        lif_op, out=msl(s0, K), in0=msl(s0 - 1, K),
                        in1=xb[:, s0 * FD:(s0 + K) * FD],
                        s0=float(c2), s1=float(d))
                    last._wait_ge(xc[b], 16)
                last.then_inc(mrd, 1)

        @blk.gpsimd
        def _(gp):
            # tiny bias const for ACT's Sign stream, off the big load queues
            gp.dma_start(cst[:, :], nb_d[:, :]).then_inc(bsm, 16)

    mybir.codegen_inst_isa_subclasses(nc)
    return nc


_PROGRAM_CACHE: dict = {}


def kernel(inpt: np.ndarray, v_th: np.ndarray, v_decay: np.ndarray) -> np.ndarray:
    global LAST_RUN
    x = np.ascontiguousarray(np.asarray(inpt, dtype=np.float32))
    assert x.shape == (T, B, N), x.shape
    vth = np.float32(np.asarray(v_th))
    d = float(np.float32(np.asarray(v_decay)))
    b, spk_is_gt = spike_boundary(vth)

    key = (float(b), d, spk_is_gt)
    if key not in _PROGRAM_CACHE:
        _PROGRAM_CACHE[key] = build_program(float(b), d, spk_is_gt)
    nc = _PROGRAM_CACHE[key]

    in_maps = []
    nbias = np.full((P, 1), -np.float32(b), dtype=np.float32)
    for k in range(NCORES):
        xk = x[:, k * BPC:(k + 1) * BPC, :].reshape(T, P, FD)
        xk = np.ascontiguousarray(xk.transpose(1, 0, 2)).reshape(P, T * FD)
        in_maps.append({"x": xk, "nbias": nbias})

    res = run_bass_kernel_spmd(
        nc, in_maps, core_ids=list(range(NCORES)), trace=_TRACE
    )
    LAST_RUN = res

    t0 = T - TAILS
    spikes = np.empty((T, B, N), dtype=np.float32)
    for k in range(NCORES):
        s = res.results[k]["m8"].astype(np.float32)
        s = s.reshape(P, T, FD).transpose(1, 0, 2).reshape(T, BPC, N)
        cmp = (s > 0) if spk_is_gt else (s < 0)
        mt = res.results[k]["mtail"].reshape(P, TAILS, FD)
        mt = mt.transpose(1, 0, 2).reshape(TAILS, BPC, N)
        cmp[t0:] = (mt > b) if spk_is_gt else (mt < b)
        spikes[:, k * BPC:(k + 1) * BPC, :] = cmp
    return spikes
